# revision 8
# baseline (speedup 1.0000x reference)
"""TRN2 Bass kernel for ChunkParallelGRU (B=64, T=512, D=H=1024, chunk=4).

Strategy: data-parallel over batch across the 8 NeuronCores (8 rows each);
no cross-core communication.  Per core:

  Phase A: Gx[(b,t), :] = x[b,t] @ [Wzx|Wrx|Whx] + [bz|br|bh] with bf16
    matmuls (x^T tiles stationary via PE transposes, weights moving),
    written to an internal DRAM buffer in natural [(b t), 3H] layout.

  Phase B: the 512-step recurrence in a transposed layout (hidden dim on
    partitions, batch on the free dim).  Weight-stationary bf16 matmuls:
    per step 3 gates x 64 [128x128] weight tiles with N=8 moving columns;
    the tanh(Wp h + bp) state propagation every 4th step.  Elementwise on
    [128, 64]-packed tiles; Gx streamed in per chunk; ys streamed out per
    step into a mirror-layout staging buffer, relayouted at the end.
    All fp32 except matmul operands (bf16).

The 512 steps run as a hardware For_i loop over 8-step bodies plus an
unrolled 16-step epilogue.
"""
import sys
sys.path.insert(0, '/opt/trn_rl_repo')
from contextlib import ExitStack

import numpy as np
import ml_dtypes

import concourse.bass as bass
from concourse import bacc
import concourse.tile as tile
from concourse import mybir
from concourse.bass import ds
from concourse.bass_utils import run_bass_kernel_spmd
from concourse.masks import make_identity

dt = mybir.dt
bf16 = ml_dtypes.bfloat16
AF = mybir.ActivationFunctionType

B, T, D, H = 64, 512, 1024, 1024
NCORES = 8
BL = B // NCORES          # 8 local batch rows
KT = D // 128             # 8 contraction tiles
MT = H // 128             # 8 output tiles per gate
G3 = 3 * H                # 3072 gate columns

_CACHE = {}


def _build(t_total=T):
    nc = bacc.Bacc("TRN2", target_bir_lowering=False, debug=False)

    x_d = nc.dram_tensor("x", [BL * t_total, D], dt.bfloat16, kind="ExternalInput").ap()
    h0_d = nc.dram_tensor("h0", [BL, H], dt.float32, kind="ExternalInput").ap()
    # phase A weights, packed [128, g*8192 + k*1024 + m]
    wx_d = nc.dram_tensor("wx", [128, 3 * KT * H], dt.bfloat16, kind="ExternalInput").ap()
    # recurrent weights, packed [128, (k*MT + m)*128 + j]
    whz_d = nc.dram_tensor("whz", [128, KT * MT * 128], dt.bfloat16, kind="ExternalInput").ap()
    whr_d = nc.dram_tensor("whr", [128, KT * MT * 128], dt.bfloat16, kind="ExternalInput").ap()
    whh_d = nc.dram_tensor("whh", [128, KT * MT * 128], dt.bfloat16, kind="ExternalInput").ap()
    wp_d = nc.dram_tensor("wp", [128, KT * MT * 128], dt.bfloat16, kind="ExternalInput").ap()
    bias_d = nc.dram_tensor("bias", [128, G3], dt.float32, kind="ExternalInput").ap()
    bpb_d = nc.dram_tensor("bpb", [128, MT * BL], dt.float32, kind="ExternalInput").ap()
    ys_d = nc.dram_tensor("ys", [BL, t_total, H], dt.float32, kind="ExternalOutput").ap()

    with tile.TileContext(nc) as tc, ExitStack() as ctx:
        dram_pool = ctx.enter_context(tc.tile_pool(name="dram", bufs=1, space="DRAM"))
        gx_tile = dram_pool.tile([BL * t_total, G3], dt.float32, tag="gx")
        yst_tile = dram_pool.tile([t_total, MT * BL * 128], dt.float32, tag="yst")
        gx_d = gx_tile[:]
        yst_d = yst_tile[:]

        # ---------------- phase A ----------------
        hh = 128 if t_total >= 128 else t_total
        n_rt = (BL * t_total) // hh
        with ExitStack() as actx:
            consts = actx.enter_context(tc.tile_pool(name="pa_consts", bufs=1))
            wx_sb = consts.tile([128, 3 * KT * H], dt.bfloat16, tag="wx")
            nc.sync.dma_start(wx_sb[:], wx_d[:])
            bias_sb = consts.tile([128, G3], dt.float32, tag="bias")
            nc.sync.dma_start(bias_sb[:], bias_d[:])
            ident = consts.tile([128, 128], dt.bfloat16, tag="ident")
            make_identity(nc, ident[:])

            xa_pool = actx.enter_context(tc.tile_pool(name="pa_xa", bufs=3))
            xt_pool = actx.enter_context(tc.tile_pool(name="pa_xt", bufs=3))
            tp_pool = actx.enter_context(tc.tile_pool(name="pa_tp", bufs=2, space="PSUM"))
            mm_pool = actx.enter_context(tc.tile_pool(name="pa_mm", bufs=4, space="PSUM"))
            go_pool = actx.enter_context(tc.tile_pool(name="pa_go", bufs=4))

            for rt in range(n_rt):        # 128-row tiles over all (b, t) rows
                r0 = rt * hh
                xa = xa_pool.tile([hh, D], dt.bfloat16, tag="xa")
                nc.sync.dma_start(xa[:], x_d[r0:r0 + hh, :])
                xt = xt_pool.tile([128, KT * hh], dt.bfloat16, tag="xt")
                for k in range(KT):
                    pst = tp_pool.tile([128, hh], dt.bfloat16, tag="pst")
                    nc.tensor.transpose(pst[:], xa[:, k * 128:(k + 1) * 128], ident[:hh, :hh])
                    nc.vector.tensor_copy(xt[:, k * hh:(k + 1) * hh], pst[:])
                for mc in range(6):       # 512-col chunks over 3*1024 gate cols
                    g, m0 = divmod(mc * 512, H)
                    ps = mm_pool.tile([hh, 512], dt.float32, tag="ps")
                    for k in range(KT):
                        nc.tensor.matmul(
                            ps[:],
                            xt[:, k * hh:(k + 1) * hh],
                            wx_sb[:, g * (KT * H) + k * H + m0:g * (KT * H) + k * H + m0 + 512],
                            start=(k == 0), stop=(k == KT - 1))
                    gout = go_pool.tile([hh, 512], dt.float32, tag="gout")
                    nc.vector.tensor_add(gout[:], ps[:], bias_sb[:hh, mc * 512:(mc + 1) * 512])
                    nc.sync.dma_start(gx_d[r0:r0 + hh, mc * 512:(mc + 1) * 512], gout[:])

        # ---------------- phase B ----------------
        rec = ctx.enter_context(tc.tile_pool(name="pb_consts", bufs=1))
        whz = rec.tile([128, KT * MT * 128], dt.bfloat16, tag="whz")
        nc.sync.dma_start(whz[:], whz_d[:])
        whr = rec.tile([128, KT * MT * 128], dt.bfloat16, tag="whr")
        nc.sync.dma_start(whr[:], whr_d[:])
        whh = rec.tile([128, KT * MT * 128], dt.bfloat16, tag="whh")
        nc.sync.dma_start(whh[:], whh_d[:])
        wp = rec.tile([128, KT * MT * 128], dt.bfloat16, tag="wp")
        nc.sync.dma_start(wp[:], wp_d[:])
        bpb = rec.tile([128, MT * BL], dt.float32, tag="bpb")
        nc.sync.dma_start(bpb[:], bpb_d[:])

        hprop = rec.tile([128, MT * BL], dt.float32, tag="hprop")
        hpropb = rec.tile([128, MT * BL], dt.bfloat16, tag="hpropb")
        gxA = rec.tile([128, 4 * 192], dt.float32, tag="gxA")
        gxB = rec.tile([128, 4 * 192], dt.float32, tag="gxB")

        # init h^T from h0: one strided DMA per m-tile (partition <- contiguous H)
        h0_t = h0_d.rearrange("b (mt p) -> p mt b", p=128)
        for mt in range(MT):
            nc.sync.dma_start(hprop[:, mt * BL:(mt + 1) * BL], h0_t[:, mt, :])
        nc.vector.tensor_copy(hpropb[:], hprop[:])

        gx_ap = gx_d.rearrange("(b t) (gm p) -> p b t gm", b=BL, p=128)   # [128,8,T,24]
        yst_ap = yst_d.rearrange("t (c p) -> p t c", p=128)               # [128,T,64]

        # per-step pools
        pz_pool = ctx.enter_context(tc.tile_pool(name="pb_pz", bufs=2, space="PSUM"))
        pr_pool = ctx.enter_context(tc.tile_pool(name="pb_pr", bufs=2, space="PSUM"))
        pc_pool = ctx.enter_context(tc.tile_pool(name="pb_pc", bufs=2, space="PSUM"))
        pw_pool = ctx.enter_context(tc.tile_pool(name="pb_pw", bufs=2, space="PSUM"))
        sb_pool = ctx.enter_context(tc.tile_pool(name="pb_sb", bufs=2))
        hn_pool = ctx.enter_context(tc.tile_pool(name="pb_hn", bufs=4))
        hb_pool = ctx.enter_context(tc.tile_pool(name="pb_hb", bufs=3))

        def mm_gate(psum, w_sb, rhs_bf):
            for mt in range(MT):
                for k in range(KT):
                    nc.tensor.matmul(
                        psum[:, mt * BL:(mt + 1) * BL],
                        w_sb[:, (k * MT + mt) * 128:(k * MT + mt) * 128 + 128],
                        rhs_bf[:, k * BL:(k + 1) * BL],
                        start=(k == 0), stop=(k == KT - 1))

        def emit_step(t_idx, gx_tile, s, h_f, h_b, prop):
            """one recurrence step; returns (h_f, h_b) for the next step"""
            gx_t = gx_tile[:, s * 192:(s + 1) * 192]
            pz = pz_pool.tile([128, MT * BL], dt.float32, tag="pz")
            mm_gate(pz, whz, h_b)
            pr = pr_pool.tile([128, MT * BL], dt.float32, tag="pr")
            mm_gate(pr, whr, h_b)

            tz = sb_pool.tile([128, MT * BL], dt.float32, tag="tz")
            nc.vector.tensor_add(tz[:], pz[:], gx_t[:, 0:64])
            zf = sb_pool.tile([128, MT * BL], dt.float32, tag="zf")
            nc.scalar.activation(zf[:], tz[:], AF.Sigmoid)
            tr = sb_pool.tile([128, MT * BL], dt.float32, tag="tr")
            nc.vector.tensor_add(tr[:], pr[:], gx_t[:, 64:128])
            rf = sb_pool.tile([128, MT * BL], dt.float32, tag="rf")
            nc.scalar.activation(rf[:], tr[:], AF.Sigmoid)

            rh = sb_pool.tile([128, MT * BL], dt.bfloat16, tag="rh")
            nc.vector.tensor_mul(rh[:], rf[:], h_f[:])

            pc = pc_pool.tile([128, MT * BL], dt.float32, tag="pc")
            mm_gate(pc, whh, rh)
            thc = sb_pool.tile([128, MT * BL], dt.float32, tag="thc")
            nc.vector.tensor_add(thc[:], pc[:], gx_t[:, 128:192])
            hcf = sb_pool.tile([128, MT * BL], dt.float32, tag="hcf")
            nc.scalar.activation(hcf[:], thc[:], AF.Tanh)

            dd = sb_pool.tile([128, MT * BL], dt.float32, tag="dd")
            nc.vector.tensor_sub(dd[:], hcf[:], h_f[:])
            zd = sb_pool.tile([128, MT * BL], dt.float32, tag="zd")
            nc.vector.tensor_mul(zd[:], zf[:], dd[:])
            hn = hn_pool.tile([128, MT * BL], dt.float32, tag="hn")
            nc.vector.tensor_add(hn[:], h_f[:], zd[:])

            nc.sync.dma_start(
                yst_ap[:, ds(t_idx, 1), :],
                hn[:].rearrange("p (o c) -> p o c", o=1))

            hb = hb_pool.tile([128, MT * BL], dt.bfloat16, tag="hb")
            nc.vector.tensor_copy(hb[:], hn[:])
            if not prop:
                return hn, hb
            # chunk boundary: h <- tanh(Wp @ h_new + bp)
            pw = pw_pool.tile([128, MT * BL], dt.float32, tag="pw")
            mm_gate(pw, wp, hb)
            tw = sb_pool.tile([128, MT * BL], dt.float32, tag="tw")
            nc.vector.tensor_add(tw[:], pw[:], bpb[:])
            nc.scalar.activation(hprop[:], tw[:], AF.Tanh)
            nc.vector.tensor_copy(hpropb[:], hprop[:])
            return hprop, hpropb

        def emit_chunk(tbase, gx_tile, prop):
            h_f, h_b = hprop, hpropb
            for s in range(4):
                h_f, h_b = emit_step(tbase + s, gx_tile, s, h_f, h_b,
                                     prop=(s == 3 and prop))

        def dma_gx(tile_, tb):
            dst = tile_[:].rearrange("p (t gm b) -> p b t gm", t=4, b=BL)
            for b in range(BL):
                nc.sync.dma_start(dst[:, b:b + 1, :, :],
                                  gx_ap[:, b:b + 1, ds(tb, 4), :])

        # prologue: load gx for chunk 0
        dma_gx(gxA, 0)

        LOOP_END = t_total - 16     # two chunk-pairs are unrolled as epilogue
        with tc.For_i(0, LOOP_END, 8, hint_engines=(mybir.EngineType.PE,)) as i:
            dma_gx(gxB, i + 4)
            emit_chunk(i, gxA, prop=True)
            dma_gx(gxA, i + 8)
            emit_chunk(i + 4, gxB, prop=True)

        # epilogue: last 16 steps (no prop after the final step)
        dma_gx(gxB, LOOP_END + 4)
        emit_chunk(LOOP_END, gxA, prop=True)
        dma_gx(gxA, LOOP_END + 8)
        emit_chunk(LOOP_END + 4, gxB, prop=True)
        dma_gx(gxB, LOOP_END + 12)
        emit_chunk(LOOP_END + 8, gxA, prop=True)
        emit_chunk(LOOP_END + 12, gxB, prop=False)

        # relayout ys_stage [t][(mt b)*128+p] -> ys [b][t][mt*128+p]
        for b in range(BL):
            for mt in range(MT):
                c = (mt * BL + b) * 128
                nc.sync.dma_start(ys_d[b:b + 1, :, mt * 128:(mt + 1) * 128],
                                  yst_d[:, c:c + 128])

    nc.compile()
    return nc


def _pack_w_tiles(W):
    """[H, H] -> [128, (k*MT+m)*128+j] stationary-tile packing (bf16)."""
    return np.ascontiguousarray(
        W.reshape(KT, 128, MT, 128).transpose(1, 0, 2, 3).reshape(128, KT * MT * 128)
    ).astype(bf16)


def _pack_w_moving(W):
    """[D, H] -> [128, k*H + m] moving packing (bf16)."""
    return np.ascontiguousarray(
        W.reshape(KT, 128, H).transpose(1, 0, 2).reshape(128, KT * H)
    ).astype(bf16)


def make_in_maps(x, h0, Wz, bz, Wr, br, Wh, bh, Wp, bp, t_total=T):
    x = np.asarray(x, np.float32)
    h0 = np.asarray(h0, np.float32)
    Wz, Wr, Wh, Wp = (np.asarray(a, np.float32) for a in (Wz, Wr, Wh, Wp))
    bz, br, bh, bp = (np.asarray(a, np.float32) for a in (bz, br, bh, bp))

    wx = np.concatenate([_pack_w_moving(Wz[:D]), _pack_w_moving(Wr[:D]),
                         _pack_w_moving(Wh[:D])], axis=1)
    whz = _pack_w_tiles(Wz[D:])
    whr = _pack_w_tiles(Wr[D:])
    whh = _pack_w_tiles(Wh[D:])
    wpp = _pack_w_tiles(Wp)
    bias = np.tile(np.concatenate([bz, br, bh])[None, :], (128, 1)).astype(np.float32)
    bpb = np.repeat(bp.reshape(MT, 128).T[:, :, None], BL, axis=2).reshape(128, MT * BL)
    bpb = np.ascontiguousarray(bpb).astype(np.float32)

    xb = x.astype(bf16)
    in_maps = []
    for c in range(NCORES):
        in_maps.append({
            "x": np.ascontiguousarray(xb[c * BL:(c + 1) * BL].reshape(BL * t_total, D)),
            "h0": np.ascontiguousarray(h0[c * BL:(c + 1) * BL]),
            "wx": wx, "whz": whz, "whr": whr, "whh": whh, "wp": wpp,
            "bias": bias, "bpb": bpb,
        })
    return in_maps


def kernel(x, h0, Wz, bz, Wr, br, Wh, bh, Wp, bp):
    if "nc" not in _CACHE:
        _CACHE["nc"] = _build(T)
    nc = _CACHE["nc"]
    in_maps = make_in_maps(x, h0, Wz, bz, Wr, br, Wh, bh, Wp, bp, T)
    res = run_bass_kernel_spmd(nc, in_maps, list(range(NCORES)))
    out = np.empty((B, T, H), np.float32)
    for c in range(NCORES):
        out[c * BL:(c + 1) * BL] = res.results[c]["ys"]
    return out
